# revision 6
# baseline (speedup 1.0000x reference)
"""Trainium2 Bass kernel for nn_LlamaMoDDecoderLayer (Mixture-of-Depths).

Strategy (8 NeuronCores, tensor-parallel, feature-major layouts):
  - Host computes the two router argmax masks in fp64 (margins >= 7e-4, so
    exact), then PERMUTES tokens: MLP-active tokens first, inactive after
    (each block keeps original order). The MLP block (gate/up/down, its two
    collectives, norm2) then runs on only ceil(|active|/512)*512 columns
    (~half) -- the dominant FLOP term of the layer.
  - Attention still runs on all tokens (every token is a key). Causality in
    permuted space is handled per (q-chunk, k-tile): host classifies each
    pair full / skip / masked and ships bf16 mask tiles for the boundary.
    Total computed score tiles ~= the causal 40/64 of the original order.
  - All activations transposed: X^T [feature, token]; every matmul contracts
    over partitions with no activation transposes.
  - Phase 1 streams hsT fp32 ONCE: squares reduced via ones-matmul on PE,
    bf16 copy kept in SBUF, then scaled in place by 1/rms (saves a second
    16.8 MB pass). Residual slice is a separate per-core input.
  - Attention: heads sharded 2/core; transposed-scores softmax with
    denominator via ones-matmul; 1/sqrt(Dh) folded into the Exp activation
    scale; per-core head context AllGathered (bf16); Wo column-sharded.
  - MLP: w_gate/w_up column-sharded, w_down row-sharded; bf16 partial
    outputs summed by per-512-column-chunk ReduceScatter so the first RS
    overlaps the next chunk's down-proj compute.
  - Matmuls bf16 (host-cast weights), fp32 PSUM accumulation.
  - Host un-permutes the output columns at the end (free).
"""

import numpy as np
import ml_dtypes

import concourse.bass as bass
import concourse.bacc as bacc
import concourse.mybir as mybir
import concourse.tile as tile
from concourse.alu_op_type import AluOpType
from concourse.bass_utils import run_bass_kernel_spmd

F32 = mybir.dt.float32
BF16 = mybir.dt.bfloat16
AF = mybir.ActivationFunctionType

S, D, H, Dh, F = 2048, 2048, 16, 128, 8192
NC = 8
HPC = H // NC            # heads per core (2)
DCC = D // NC            # output cols per core (256)
FPC = F // NC            # mlp hidden per core (1024)
NDT = D // 128           # 16 d-tiles
NFT = FPC // 128         # 8 local f-tiles
NKT = S // 128           # 16 k-tiles
EPS = 1e-5
THETA = 10000.0

_CACHE = {}


def _host_plan(inputs):
    """fp64 router masks -> token permutation + attention tile plan."""
    hs = np.asarray(inputs["hidden_states"], np.float64)[0]        # [S, D]
    la = hs @ np.asarray(inputs["router_attn_w"], np.float64) \
        + np.asarray(inputs["router_attn_b"], np.float64)
    lm = hs @ np.asarray(inputs["router_mlp_w"], np.float64) \
        + np.asarray(inputs["router_mlp_b"], np.float64)
    route_attn = np.argmax(la, axis=-1).astype(bool)   # True -> attn zeroed
    mlp_active = ~np.argmax(lm, axis=-1).astype(bool)  # True -> mlp applied
    pi = np.concatenate([np.nonzero(mlp_active)[0],
                         np.nonzero(~mlp_active)[0]]).astype(np.int64)
    SM = int(mlp_active.sum())

    # q-chunks: split each block into <=512-wide chunks (block-aligned so a
    # chunk never spans the permutation discontinuity).
    qchunks = []
    for lo, hi in ((0, SM), (SM, S)):
        n = max(1, -(-(hi - lo) // 512))
        if hi > lo:
            for part in np.array_split(np.arange(lo, hi), n):
                qchunks.append((int(part[0]), int(len(part))))

    # classify (q-chunk, k-tile): skip / full / masked
    klists = []
    mask_tiles = []
    for (qs, ql) in qchunks:
        opq = pi[qs:qs + ql]
        qmin, qmax = opq.min(), opq.max()
        kl = []
        for kt in range(NKT):
            opk = pi[kt * 128:(kt + 1) * 128]
            if opk.min() > qmax:
                continue                       # fully masked: skip
            if opk.max() <= qmin:
                kl.append((kt, -1))            # fully valid
            else:
                m = np.zeros((128, 512), np.float32)
                m[:, :ql] = (opk[:, None] <= opq[None, :])
                kl.append((kt, len(mask_tiles)))
                mask_tiles.append(m)
        klists.append(kl)
    n_mask = len(mask_tiles)
    masks = (np.stack(mask_tiles, axis=1).reshape(128, n_mask * 512)
             if n_mask else np.zeros((128, 512), np.float32))

    ma_row = np.where(route_attn, 0.0, 1.0)[pi].astype(np.float32)[None, :]
    return dict(pi=pi, SM=SM, qchunks=tuple(qchunks),
                klists=tuple(tuple(kl) for kl in klists),
                n_mask=n_mask, masks=masks, ma_row=ma_row)


def _build_program(SM, qchunks, klists, n_mask):
    SM_pad = max(512, -(-SM // 512) * 512)
    NMC = SM_pad // 512          # mlp 512-col chunks (collective-chunked)

    nc = bacc.Bacc("TRN2", target_bir_lowering=False, debug=False,
                   num_devices=NC)
    rg = [list(range(NC))]

    d_hsT = nc.dram_tensor("hsT", [D, S], F32, kind="ExternalInput")
    d_hres = nc.dram_tensor("hres", [DCC, S], F32, kind="ExternalInput")
    d_wq = nc.dram_tensor("wq", [D, DCC], BF16, kind="ExternalInput")
    d_wk = nc.dram_tensor("wk", [D, DCC], BF16, kind="ExternalInput")
    d_wv = nc.dram_tensor("wv", [D, DCC], BF16, kind="ExternalInput")
    d_wo = nc.dram_tensor("wo", [D, DCC], BF16, kind="ExternalInput")
    d_wg = nc.dram_tensor("wg", [D, FPC], BF16, kind="ExternalInput")
    d_wu = nc.dram_tensor("wu", [D, FPC], BF16, kind="ExternalInput")
    d_wd = nc.dram_tensor("wd", [FPC, D], BF16, kind="ExternalInput")
    d_cos = nc.dram_tensor("cos", [Dh, S], BF16, kind="ExternalInput")
    d_sin = nc.dram_tensor("sin", [Dh, S], BF16, kind="ExternalInput")
    d_masks = nc.dram_tensor("masks", [128, max(n_mask, 1) * 512], BF16,
                             kind="ExternalInput")
    d_ma = nc.dram_tensor("ma", [1, S], F32, kind="ExternalInput")
    d_out = nc.dram_tensor("out", [DCC, S], F32, kind="ExternalOutput")

    cc1_in = nc.dram_tensor("cc1_in", [DCC, S], BF16)
    cc1_out = nc.dram_tensor("cc1_out", [D, S], BF16, addr_space="Shared")
    cc2_in = nc.dram_tensor("cc2_in", [DCC, SM_pad], BF16)
    cc2_out = nc.dram_tensor("cc2_out", [D, SM_pad], BF16,
                             addr_space="Shared")
    cc3_in = [nc.dram_tensor(f"cc3_in{j}", [D, 512], BF16)
              for j in range(NMC)]
    cc3_out = [nc.dram_tensor(f"cc3_out{j}", [DCC, 512], BF16)
               for j in range(NMC)]

    hsT_t = d_hsT.ap().rearrange("(a p) s -> p a s", p=128)
    hres_t = d_hres.ap().rearrange("(a p) s -> p a s", p=128)
    wq_t = d_wq.ap().rearrange("(a p) m -> p a m", p=128)
    wk_t = d_wk.ap().rearrange("(a p) m -> p a m", p=128)
    wv_t = d_wv.ap().rearrange("(a p) m -> p a m", p=128)
    wo_t = d_wo.ap().rearrange("(a p) m -> p a m", p=128)
    wg_t = d_wg.ap().rearrange("(a p) m -> p a m", p=128)
    wu_t = d_wu.ap().rearrange("(a p) m -> p a m", p=128)
    wd_t = d_wd.ap().rearrange("(a p) m -> p a m", p=128)
    masks_t = d_masks.ap().rearrange("p (n m) -> p n m", m=512)
    cc1i_t = cc1_in.ap().rearrange("(a p) s -> p a s", p=128)
    cc1o_t = cc1_out.ap().rearrange("(a p) s -> p a s", p=128)
    cc2i_t = cc2_in.ap().rearrange("(a p) s -> p a s", p=128)
    cc2o_t = cc2_out.ap().rearrange("(a p) s -> p a s", p=128)
    cc3i_t = [t.ap().rearrange("(a p) s -> p a s", p=128) for t in cc3_in]
    cc3o_t = [t.ap().rearrange("(a p) s -> p a s", p=128) for t in cc3_out]
    out_t = d_out.ap().rearrange("(a p) s -> p a s", p=128)

    sc_dh = float(1.0 / np.sqrt(Dh))

    with tile.TileContext(nc) as tc:
        with (
            tc.tile_pool(name="const", bufs=1) as cst,
            tc.tile_pool(name="resid", bufs=1) as rsp,
            tc.tile_pool(name="psum", bufs=2, space="PSUM") as psp,
        ):
            ones_b = cst.tile([128, 1], BF16)
            nc.gpsimd.memset(ones_b[:], 1.0)
            ones_r = cst.tile([1, 128], F32)
            nc.gpsimd.memset(ones_r[:], 1.0)
            ones_f = cst.tile([128, 1], F32)
            nc.gpsimd.memset(ones_f[:], 1.0)
            eps1 = cst.tile([1, 1], F32)
            nc.gpsimd.memset(eps1[:], EPS)
            cosb = cst.tile([128, S], BF16, name="cosb")
            sinb = cst.tile([128, S], BF16, name="sinb")
            nc.sync.dma_start(cosb[:], d_cos.ap())
            nc.sync.dma_start(sinb[:], d_sin.ap())
            ma_row = cst.tile([1, S], F32, name="ma_row")
            nc.sync.dma_start(ma_row[:], d_ma.ap())

            hres = rsp.tile([128, 2, S], F32, name="hres")
            nc.sync.dma_start(hres[:], hres_t)
            hs2f = rsp.tile([128, 2, S], F32, name="hs2f")

            with tc.tile_pool(name="attn", bufs=1) as atp:
                qr = atp.tile([128, HPC, S], BF16, name="qr")
                kr = atp.tile([128, HPC, S], BF16, name="kr")
                v_sb = atp.tile([128, NDT, DCC], BF16, name="v_sb")
                ctxT = atp.tile([128, HPC, S], BF16, name="ctxT")

                with tc.tile_pool(name="xn", bufs=1) as xnp:
                    xnT = xnp.tile([128, NDT, S], BF16, name="xnT")

                    # ---- phase 1: stream hsT once; norm1 -> xnT ----
                    with tc.tile_pool(name="ph1", bufs=1) as p1:
                        r1row = p1.tile([1, S], F32, name="r1row")
                        r1b = p1.tile([128, S], F32, name="r1b")
                        acc = p1.tile([128, S], F32, name="acc")
                        for a in range(NDT):
                            ht = p1.tile([128, S], F32, tag="hst", bufs=2)
                            nc.sync.dma_start(ht[:], hsT_t[:, a, :])
                            sqt = p1.tile([128, S], BF16, tag="sq", bufs=2)
                            nc.scalar.activation(sqt[:], ht[:], AF.Square)
                            nc.vector.tensor_copy(xnT[:, a, :], ht[:])
                            if a == 0:
                                nc.vector.tensor_copy(acc[:], sqt[:])
                            else:
                                nc.vector.tensor_tensor(acc[:], acc[:],
                                                        sqt[:],
                                                        op=AluOpType.add)
                        for sc in range(4):
                            rp = psp.tile([1, 512], F32, tag="rowps")
                            nc.tensor.matmul(rp[:], ones_f[:],
                                             acc[:, bass.ts(sc, 512)])
                            nc.scalar.activation(r1row[:, bass.ts(sc, 512)],
                                                 rp[:], AF.Sqrt,
                                                 bias=eps1[:], scale=1.0 / D)
                            nc.vector.reciprocal(r1row[:, bass.ts(sc, 512)],
                                                 r1row[:, bass.ts(sc, 512)])
                            bcp = psp.tile([128, 512], F32, tag="mmps")
                            nc.tensor.matmul(bcp[:], ones_r[:],
                                             r1row[:, bass.ts(sc, 512)])
                            nc.scalar.copy(r1b[:, bass.ts(sc, 512)], bcp[:])
                        for a in range(NDT):
                            nc.vector.tensor_tensor(
                                xnT[:, a, :], xnT[:, a, :], r1b[:],
                                op=AluOpType.mult)

                    # ---- phase 2: QKV + rope ----
                    with tc.tile_pool(name="qkv", bufs=1) as qkp:
                        wq = qkp.tile([128, NDT, DCC], BF16, name="wq")
                        wk = qkp.tile([128, NDT, DCC], BF16, name="wk")
                        wv = qkp.tile([128, NDT, DCC], BF16, name="wv")
                        nc.sync.dma_start(wq[:], wq_t)
                        nc.sync.dma_start(wk[:], wk_t)
                        nc.sync.dma_start(wv[:], wv_t)
                        q_sb = qkp.tile([128, HPC, S], BF16, name="q_sb")
                        k_sb = qkp.tile([128, HPC, S], BF16, name="k_sb")
                        for w_sb, t_sb in ((wq, q_sb), (wk, k_sb)):
                            for mc in range(HPC):
                                for sc in range(4):
                                    ps = psp.tile([128, 512], F32, tag="mmps")
                                    for a in range(NDT):
                                        nc.tensor.matmul(
                                            ps[:],
                                            w_sb[:, a, bass.ts(mc, 128)],
                                            xnT[:, a, bass.ts(sc, 512)],
                                            start=(a == 0),
                                            stop=(a == NDT - 1))
                                    nc.scalar.copy(
                                        t_sb[:, mc, bass.ts(sc, 512)], ps[:])
                        for mc in range(NDT):
                            ps = psp.tile([128, DCC], F32, tag="mmps")
                            for a in range(NDT):
                                nc.tensor.matmul(ps[:],
                                                 xnT[:, a, bass.ts(mc, 128)],
                                                 wv[:, a, :],
                                                 start=(a == 0),
                                                 stop=(a == NDT - 1))
                            nc.scalar.copy(v_sb[:, mc, :], ps[:])
                        # rope: qr = q*cos + swap(q)*sin (sin pre-negated on
                        # its first 64 rows host-side)
                        for src, dst in ((q_sb, qr), (k_sb, kr)):
                            for mc in range(HPC):
                                sw = qkp.tile([128, S], BF16, tag="ropesw",
                                              bufs=2)
                                nc.sync.dma_start(sw[0:64, :],
                                                  src[64:128, mc, :])
                                nc.sync.dma_start(sw[64:128, :],
                                                  src[0:64, mc, :])
                                tq = qkp.tile([128, S], BF16, tag="ropetmp",
                                              bufs=2)
                                nc.vector.tensor_tensor(tq[:], sw[:], sinb[:],
                                                        op=AluOpType.mult)
                                nc.vector.tensor_tensor(
                                    dst[:, mc, :], src[:, mc, :], cosb[:],
                                    op=AluOpType.mult)
                                nc.vector.tensor_tensor(
                                    dst[:, mc, :], dst[:, mc, :], tq[:],
                                    op=AluOpType.add)

                # ---- phase 3: attention (permuted-causal tiles) ----
                with tc.tile_pool(name="att3", bufs=1) as a3p:
                    for h in range(HPC):
                        for ci, (qs, ql) in enumerate(qchunks):
                            kl = klists[ci]
                            nkt = len(kl)
                            cps = psp.tile([128, ql], F32, tag="ctxps",
                                           bufs=1)
                            dps = psp.tile([1, ql], F32, tag="rowps", bufs=2)
                            for i, (kt, mi) in enumerate(kl):
                                sps = psp.tile([128, ql], F32, tag="stps")
                                nc.tensor.matmul(
                                    sps[:], kr[:, h, bass.ts(kt, 128)],
                                    qr[:, h, bass.ds(qs, ql)])
                                est = a3p.tile([128, ql], BF16, tag="est",
                                               bufs=3)
                                nc.scalar.activation(est[:], sps[:], AF.Exp,
                                                     scale=sc_dh)
                                if mi >= 0:
                                    mt = a3p.tile([128, 512], BF16,
                                                  tag="mask", bufs=3)
                                    nc.sync.dma_start(mt[:],
                                                      masks_t[:, mi, :])
                                    nc.vector.tensor_tensor(
                                        est[:], est[:], mt[:, 0:ql],
                                        op=AluOpType.mult)
                                nc.tensor.matmul(cps[:],
                                                 v_sb[:, kt, bass.ts(h, 128)],
                                                 est[:], start=(i == 0),
                                                 stop=(i == nkt - 1))
                                nc.tensor.matmul(dps[:], ones_b[:], est[:],
                                                 start=(i == 0),
                                                 stop=(i == nkt - 1))
                            rrow = a3p.tile([1, ql], F32, tag="rrow", bufs=2)
                            nc.vector.reciprocal(rrow[:], dps[:])
                            rb = a3p.tile([128, ql], F32, tag="rb", bufs=2)
                            nc.gpsimd.partition_broadcast(rb[:], rrow[:])
                            nc.vector.tensor_tensor(
                                ctxT[:, h, bass.ds(qs, ql)], cps[:], rb[:],
                                op=AluOpType.mult)
                    for mc in range(HPC):
                        nc.sync.dma_start(cc1i_t[:, mc, :], ctxT[:, mc, :])

            # ---- phase 4: AG ctx + Wo proj + hs2 ----
            nc.gpsimd.collective_compute(
                "AllGather", AluOpType.bypass, replica_groups=rg,
                ins=[cc1_in.ap()], outs=[cc1_out.ap()])
            with tc.tile_pool(name="wo_ph", bufs=1) as wop:
                ctxg = wop.tile([128, NDT, S], BF16, name="ctxg")
                for a in range(NDT):
                    nc.sync.dma_start(ctxg[:, a, :], cc1o_t[:, a, :])
                wo = wop.tile([128, NDT, DCC], BF16, name="wo")
                nc.sync.dma_start(wo[:], wo_t)
                ma_b = wop.tile([128, S], F32, name="ma_b")
                for sc in range(4):
                    mbp = psp.tile([128, 512], F32, tag="mmps")
                    nc.tensor.matmul(mbp[:], ones_r[:],
                                     ma_row[:, bass.ts(sc, 512)])
                    nc.scalar.copy(ma_b[:, bass.ts(sc, 512)], mbp[:])
                hs2b = wop.tile([128, 2, S], BF16, name="hs2b")
                for mc in range(HPC):
                    for sc in range(4):
                        ps = psp.tile([128, 512], F32, tag="mmps")
                        for a in range(NDT):
                            nc.tensor.matmul(
                                ps[:], wo[:, a, bass.ts(mc, 128)],
                                ctxg[:, a, bass.ts(sc, 512)],
                                start=(a == 0), stop=(a == NDT - 1))
                        t = wop.tile([128, 512], F32, tag="wot", bufs=2)
                        nc.vector.tensor_tensor(
                            t[:], ps[:], ma_b[:, bass.ts(sc, 512)],
                            op=AluOpType.mult)
                        nc.vector.tensor_tensor(
                            hs2f[:, mc, bass.ts(sc, 512)], t[:],
                            hres[:, mc, bass.ts(sc, 512)],
                            op=AluOpType.add)
                        nc.scalar.copy(hs2b[:, mc, bass.ts(sc, 512)],
                                       hs2f[:, mc, bass.ts(sc, 512)])
                for mc in range(HPC):
                    nc.sync.dma_start(cc2i_t[:, mc, :],
                                      hs2b[:, mc, 0:SM_pad])
            nc.gpsimd.collective_compute(
                "AllGather", AluOpType.bypass, replica_groups=rg,
                ins=[cc2_in.ap()], outs=[cc2_out.ap()])

            # ---- phase 5: norm2 + MLP on the SM_pad active columns ----
            with tc.tile_pool(name="mlp", bufs=1) as mlp:
                hs2g = mlp.tile([128, NDT, SM_pad], BF16, name="hs2g")
                for a in range(NDT):
                    nc.sync.dma_start(hs2g[:, a, :], cc2o_t[:, a, :])
                with tc.tile_pool(name="r2p", bufs=1) as r2p:
                    r2row = r2p.tile([1, SM_pad], F32, name="r2row")
                    r2b = r2p.tile([128, SM_pad], F32, name="r2b")
                    for sc in range(NMC):
                        ssp = psp.tile([1, 512], F32, tag="rowps", bufs=2)
                        for a in range(NDT):
                            sqt = r2p.tile([128, 512], BF16, tag="sq2",
                                           bufs=3)
                            nc.scalar.activation(
                                sqt[:], hs2g[:, a, bass.ts(sc, 512)],
                                AF.Square)
                            nc.tensor.matmul(ssp[:], ones_b[:], sqt[:],
                                             start=(a == 0),
                                             stop=(a == NDT - 1))
                        nc.scalar.activation(r2row[:, bass.ts(sc, 512)],
                                             ssp[:], AF.Sqrt,
                                             bias=eps1[:], scale=1.0 / D)
                        nc.vector.reciprocal(r2row[:, bass.ts(sc, 512)],
                                             r2row[:, bass.ts(sc, 512)])
                        bcp = psp.tile([128, 512], F32, tag="mmps")
                        nc.tensor.matmul(bcp[:], ones_r[:],
                                         r2row[:, bass.ts(sc, 512)])
                        nc.scalar.copy(r2b[:, bass.ts(sc, 512)], bcp[:])
                    for a in range(NDT):
                        nc.vector.tensor_tensor(
                            hs2g[:, a, :], hs2g[:, a, :], r2b[:],
                            op=AluOpType.mult)
                xn2 = hs2g  # normalized in place
                hT = mlp.tile([128, NFT, SM_pad], BF16, name="hT")
                with tc.tile_pool(name="wstream", bufs=3) as wsp:
                    for fc in range(NFT):
                        wgc = wsp.tile([128, NDT, 128], BF16, tag="wgc")
                        nc.sync.dma_start(wgc[:],
                                          wg_t[:, :, bass.ts(fc, 128)])
                        sg = wsp.tile([128, SM_pad], BF16, tag="sg", bufs=2)
                        for sc in range(NMC):
                            ps = psp.tile([128, 512], F32, tag="mmps")
                            for a in range(NDT):
                                nc.tensor.matmul(
                                    ps[:], wgc[:, a, :],
                                    xn2[:, a, bass.ts(sc, 512)],
                                    start=(a == 0), stop=(a == NDT - 1))
                            nc.scalar.activation(sg[:, bass.ts(sc, 512)],
                                                 ps[:], AF.Silu)
                        wuc = wsp.tile([128, NDT, 128], BF16, tag="wuc")
                        nc.sync.dma_start(wuc[:],
                                          wu_t[:, :, bass.ts(fc, 128)])
                        for sc in range(NMC):
                            ps = psp.tile([128, 512], F32, tag="mmps")
                            for a in range(NDT):
                                nc.tensor.matmul(
                                    ps[:], wuc[:, a, :],
                                    xn2[:, a, bass.ts(sc, 512)],
                                    start=(a == 0), stop=(a == NDT - 1))
                            nc.vector.tensor_tensor(
                                hT[:, fc, bass.ts(sc, 512)], ps[:],
                                sg[:, bass.ts(sc, 512)],
                                op=AluOpType.mult)
                    # down-proj, one 512-col chunk at a time; RS per chunk
                    # so chunk j's collective overlaps chunk j+1's compute
                    for j in range(NMC):
                        for mc in range(NDT):
                            wdc = wsp.tile([128, NFT, 128], BF16, tag="wdc")
                            nc.sync.dma_start(wdc[:],
                                              wd_t[:, :, bass.ts(mc, 128)])
                            ps = psp.tile([128, 512], F32, tag="mmps")
                            for a in range(NFT):
                                nc.tensor.matmul(
                                    ps[:], wdc[:, a, :],
                                    hT[:, a, bass.ts(j, 512)],
                                    start=(a == 0), stop=(a == NFT - 1))
                            stg = wsp.tile([128, 512], BF16, tag="stg",
                                           bufs=3)
                            nc.scalar.copy(stg[:], ps[:])
                            nc.sync.dma_start(cc3i_t[j][:, mc, :], stg[:])
                        nc.gpsimd.collective_compute(
                            "ReduceScatter", AluOpType.add,
                            replica_groups=rg,
                            ins=[cc3_in[j].ap()], outs=[cc3_out[j].ap()])

            # ---- phase 6: final residual (first SM cols get mlp add) ----
            with tc.tile_pool(name="fin", bufs=1) as fin:
                for j in range(NMC):
                    lo = j * 512
                    ln = min(512, SM - lo)
                    if ln <= 0:
                        break
                    rs = fin.tile([128, 2, 512], BF16, tag="rs", bufs=2)
                    for mc in range(HPC):
                        nc.sync.dma_start(rs[:, mc, :], cc3o_t[j][:, mc, :])
                    for mc in range(HPC):
                        t2 = fin.tile([128, 512], F32, tag="fint", bufs=2)
                        nc.vector.tensor_tensor(
                            t2[:, 0:ln], rs[:, mc, 0:ln],
                            hs2f[:, mc, bass.ds(lo, ln)], op=AluOpType.add)
                        nc.sync.dma_start(out_t[:, mc, bass.ds(lo, ln)],
                                          t2[:, 0:ln])
                # inactive tail: out = hs2 unchanged
                if SM < S:
                    for mc in range(HPC):
                        nc.sync.dma_start(out_t[:, mc, bass.ds(SM, S - SM)],
                                          hs2f[:, mc, bass.ds(SM, S - SM)])

    nc.compile()
    return nc


def _rope_tables(pi):
    pos = np.arange(S, dtype=np.float32)
    inv = 1.0 / (THETA ** (np.arange(0, Dh, 2, dtype=np.float32) / Dh))
    ang = pos[:, None] * inv[None, :]
    emb = np.concatenate([ang, ang], axis=-1)          # [S, Dh]
    cosT = np.cos(emb).T.astype(np.float32)            # [Dh, S]
    sinT = np.sin(emb).T.astype(np.float32)
    sinT[:64] = -sinT[:64]
    return cosT[:, pi].copy(), sinT[:, pi].copy()


def kernel(**inputs):
    bf = ml_dtypes.bfloat16
    plan = _host_plan(inputs)
    pi, SM = plan["pi"], plan["SM"]

    hs = np.ascontiguousarray(np.asarray(inputs["hidden_states"],
                                         np.float32)[0])
    ln1 = np.asarray(inputs["ln1_w"], np.float32)
    ln2 = np.asarray(inputs["ln2_w"], np.float32)
    Wq = np.asarray(inputs["Wq"], np.float32) * ln1[:, None]
    Wk = np.asarray(inputs["Wk"], np.float32) * ln1[:, None]
    Wv = np.asarray(inputs["Wv"], np.float32) * ln1[:, None]
    Wo = np.asarray(inputs["Wo"], np.float32)
    wg = np.asarray(inputs["w_gate"], np.float32) * ln2[:, None]
    wu = np.asarray(inputs["w_up"], np.float32) * ln2[:, None]
    wd = np.asarray(inputs["w_down"], np.float32)

    hsT = np.ascontiguousarray(hs.T[:, pi])            # [D, S] permuted
    cosT, sinT = _rope_tables(pi)
    cosb = np.ascontiguousarray(cosT.astype(bf))
    sinb = np.ascontiguousarray(sinT.astype(bf))
    masksb = np.ascontiguousarray(plan["masks"].astype(bf))

    sig = (SM, plan["qchunks"], plan["klists"], plan["n_mask"])
    if _CACHE.get("sig") != sig:
        _CACHE.clear()
        _CACHE["sig"] = sig
        _CACHE["nc"] = _build_program(SM, plan["qchunks"], plan["klists"],
                                      plan["n_mask"])
    nc = _CACHE["nc"]

    in_maps = []
    for c in range(NC):
        dsl = slice(c * DCC, (c + 1) * DCC)
        fsl = slice(c * FPC, (c + 1) * FPC)
        in_maps.append({
            "hsT": hsT,
            "hres": np.ascontiguousarray(hsT[dsl]),
            "wq": np.ascontiguousarray(Wq[:, dsl].astype(bf)),
            "wk": np.ascontiguousarray(Wk[:, dsl].astype(bf)),
            "wv": np.ascontiguousarray(Wv[:, dsl].astype(bf)),
            "wo": np.ascontiguousarray(Wo[:, dsl].astype(bf)),
            "wg": np.ascontiguousarray(wg[:, fsl].astype(bf)),
            "wu": np.ascontiguousarray(wu[:, fsl].astype(bf)),
            "wd": np.ascontiguousarray(wd[fsl].astype(bf)),
            "cos": cosb, "sin": sinb,
            "masks": masksb, "ma": plan["ma_row"],
        })
    _CACHE["in_maps"] = in_maps
    res = run_bass_kernel_spmd(nc, in_maps, core_ids=list(range(NC)))
    _CACHE["res"] = res
    outT = np.concatenate([res.results[c]["out"] for c in range(NC)], axis=0)
    out = np.empty((S, D), np.float32)
    out[pi] = outT.T
    return out[None]


if __name__ == "__main__":
    import reference
    inputs = reference.setup_inputs()
    out = kernel(**inputs)
    print(out.shape, out.dtype)
